# revision 5
# baseline (speedup 1.0000x reference)
"""Trainium2 Bass kernel for the audio-visual attention model.

Math (per (b,t) sample, BT = 32*64 = 2048 of them):
    V   = video[b,t]                              # [48, 512]
    v   = relu(V @ W_video.T + b_video)           # [48, 512]
    a   = relu(audio[b,t] @ W_audio.T + b_audio)  # [512]
    inter   = a @ W_g.T                           # [48]
    content = v @ W_v.T + inter[:, None]          # [48, 48]
    z   = tanh(content) @ W_h.T                   # [48]
    alpha = softmax(z)
    out = alpha @ V                               # [512]

Strategy: data-parallel over BT across 8 cores (256 samples each, R = 256*48
= 12288 video rows per core).  The host pre-transposes the video shard to
V.T [512, 12288] so the contraction dim (vsize) lands on SBUF partitions.
All matmuls run as float32r (FP22) at full PE rate (output free dim >= 256).

Per-core device pipeline, over 8 superblocks of 1536 rows (3 sub-blocks of
512 rows each):
    vT.relu  = relu(W_video.T^T @ V.T + b_video)      PE + ACT   [h, rows]
    content.T = W_v.T^T @ vT.relu (+ ones^T @ inter)  PE         [48, rows]
    tanhc    = tanh(content.T)                        ACT
    z        = W_h.T^T @ tanhc                        PE         [1, rows]
    expz     = exp(z)  (no max-sub needed: |z| <= ~4) ACT        [1, rows]
    expz_b   = ones128^T @ expz (partition broadcast) PE         [128, rows]
    weighted = V.T * expz_b  (in place)               DVE
    cT[:, groups] = segmented-sum over 48-row groups  DVE        [128, 32]
plus a tiny audio phase up front (a.T, inter, DRAM roundtrip to flatten
inter to row-major [1, 12288]), and a tail normalize by 1/sum(expz).
Output is c.T [512, 256] per core; the host transposes back.
"""

import numpy as np

# Problem constants (hardcoded per harness contract).
B, T = 32, 64
ASIZE, VSIZE, HSIZE, MSIZE = 128, 512, 512, 48
NCORES = 8
BT = B * T                     # 2048
PER = BT // NCORES             # 256 samples per core
R = PER * MSIZE                # 12288 video rows per core
SUPER = 1536                   # rows per superblock (32 groups of 48)
NSB = R // SUPER               # 8 superblocks
SUB = 512                      # matmul moving-dim block
NSUB = SUPER // SUB            # 3
GPS = SUPER // MSIZE           # 32 groups per superblock

_cached = {}


def _build_nc():
    import concourse.bacc as bacc
    import concourse.mybir as mybir
    import concourse.tile as tile

    f32 = mybir.dt.float32
    f32r = mybir.dt.float32r
    AF = mybir.ActivationFunctionType
    AX = mybir.AxisListType

    nc = bacc.Bacc(
        "TRN2",
        target_bir_lowering=False,
        debug=False,
        enable_asserts=False,
        num_devices=NCORES,
    )

    # ---- DRAM I/O ----
    vT_d = nc.dram_tensor("vT", [VSIZE, R], f32r, kind="ExternalInput").ap()
    audioT_d = nc.dram_tensor("audioT", [ASIZE, PER], f32r, kind="ExternalInput").ap()
    wvideoT_d = nc.dram_tensor("WvideoT", [VSIZE, HSIZE], f32r, kind="ExternalInput").ap()
    waudioT_d = nc.dram_tensor("WaudioT", [ASIZE, HSIZE], f32r, kind="ExternalInput").ap()
    wgT_d = nc.dram_tensor("WgT", [HSIZE, MSIZE], f32r, kind="ExternalInput").ap()
    wvT_d = nc.dram_tensor("WvT", [HSIZE, MSIZE], f32r, kind="ExternalInput").ap()
    whT_d = nc.dram_tensor("WhT", [MSIZE, 1], f32r, kind="ExternalInput").ap()
    bvideo_d = nc.dram_tensor("b_video", [HSIZE], f32, kind="ExternalInput").ap()
    baudio_d = nc.dram_tensor("b_audio", [HSIZE], f32, kind="ExternalInput").ap()
    cT_d = nc.dram_tensor("cT", [VSIZE, PER], f32, kind="ExternalOutput").ap()

    KC = VSIZE // 128          # 4 contraction chunks for the main matmul
    HC = HSIZE // 128          # 4 h chunks

    with tile.TileContext(nc) as tc:
        with (
            tc.tile_pool(name="const", bufs=1) as const,
            tc.tile_pool(name="dram", bufs=1, space="DRAM") as dramp,
        ):
            # ---- constants / weights ----
            wvideoT_sb = const.tile([128, KC, HSIZE], f32r)
            nc.sync.dma_start(
                out=wvideoT_sb, in_=wvideoT_d.rearrange("(c p) h -> p c h", p=128)
            )
            waudioT_sb = const.tile([128, HSIZE], f32r)
            nc.sync.dma_start(out=waudioT_sb, in_=waudioT_d)
            wgT_sb = const.tile([128, HC, MSIZE], f32r)
            nc.sync.dma_start(out=wgT_sb, in_=wgT_d.rearrange("(c p) m -> p c m", p=128))
            wvT_sb = const.tile([128, HC, MSIZE], f32r)
            nc.sync.dma_start(out=wvT_sb, in_=wvT_d.rearrange("(c p) m -> p c m", p=128))
            whT_sb = const.tile([MSIZE, 1], f32r)
            nc.sync.dma_start(out=whT_sb, in_=whT_d)
            bvideo_sb = const.tile([128, HC], f32)
            nc.sync.dma_start(out=bvideo_sb, in_=bvideo_d.rearrange("(c p) -> p c", p=128))
            baudio_sb = const.tile([128, HC], f32)
            nc.sync.dma_start(out=baudio_sb, in_=baudio_d.rearrange("(c p) -> p c", p=128))
            ones_f32 = const.tile([1, 128], f32)
            nc.vector.memset(ones_f32, 1.0)
            ones48 = const.tile([1, MSIZE], f32r)
            nc.vector.tensor_copy(out=ones48, in_=ones_f32[:, :MSIZE])
            ones128 = const.tile([1, 128], f32r)
            nc.vector.tensor_copy(out=ones128, in_=ones_f32)

            # persistent accumulators
            cT_acc = const.tile([128, KC, PER], f32)
            denom_sb = const.tile([1, PER], f32)
            recip_sb = const.tile([1, PER], f32r)
            inter_dram = dramp.tile([PER, MSIZE], f32r)

            # ---- audio phase: a.T = relu(W_audio.T^T @ audio.T + b_audio) ----
            with (
                tc.tile_pool(name="aud_ps", bufs=2, space="PSUM") as aud_ps,
                tc.tile_pool(name="aud_sb", bufs=1) as aud_sb,
            ):
                audioT_sb = aud_sb.tile([128, PER], f32r)
                nc.sync.dma_start(out=audioT_sb, in_=audioT_d)
                aT_sb = aud_sb.tile([128, HC, PER], f32r)
                for m in range(HC):
                    a_ps = aud_ps.tile([128, PER], f32, tag="a_ps")
                    nc.tensor.matmul(
                        a_ps,
                        waudioT_sb[:, m * 128 : (m + 1) * 128],
                        audioT_sb,
                        start=True,
                        stop=True,
                    )
                    nc.scalar.activation(
                        out=aT_sb[:, m, :], in_=a_ps, func=AF.Relu,
                        bias=baudio_sb[:, m : m + 1],
                    )
                # inter[bt, m] = a @ W_g.T, natural layout for a flat DRAM write
                inter_sb = aud_sb.tile([128, PER // 128, MSIZE], f32r)
                for t in range(PER // 128):
                    i_ps = aud_ps.tile([128, MSIZE], f32, tag="i_ps")
                    for k in range(HC):
                        nc.tensor.matmul(
                            i_ps,
                            aT_sb[:, k, t * 128 : (t + 1) * 128],
                            wgT_sb[:, k, :],
                            start=(k == 0),
                            stop=(k == HC - 1),
                        )
                    nc.scalar.copy(out=inter_sb[:, t, :], in_=i_ps)
                nc.sync.dma_start(
                    out=inter_dram.rearrange("(t p) m -> p t m", p=128), in_=inter_sb
                )

            # ---- main loop over superblocks ----
            with (
                tc.tile_pool(name="vt", bufs=2) as vtp,
                tc.tile_pool(name="vrelu", bufs=2) as vrp,
                tc.tile_pool(name="tanhp", bufs=2) as thp,
                tc.tile_pool(name="small", bufs=2) as smp,
                tc.tile_pool(name="mm_ps", bufs=2, space="PSUM") as mm_ps,
                tc.tile_pool(name="ct_ps", bufs=2, space="PSUM") as ct_ps,
                tc.tile_pool(name="z_ps", bufs=2, space="PSUM") as z_ps,
                tc.tile_pool(name="eb_ps", bufs=2, space="PSUM") as eb_ps,
            ):
                for sb in range(NSB):
                    r0 = sb * SUPER
                    vt_tile = vtp.tile([128, KC, SUPER], f32r, tag="vt")
                    nc.sync.dma_start(
                        out=vt_tile,
                        in_=vT_d[:, r0 : r0 + SUPER].rearrange("(c p) n -> p c n", p=128),
                    )
                    interflat = smp.tile([1, SUPER], f32r, tag="interflat")
                    nc.sync.dma_start(
                        out=interflat,
                        in_=inter_dram.flatten()[r0 : r0 + SUPER].unsqueeze(0),
                    )
                    vrelu = vrp.tile([128, HC, SUPER], f32r, tag="vrelu")
                    tanhc = thp.tile([MSIZE, SUPER], f32r, tag="tanhc")
                    expz = smp.tile([1, SUPER], f32r, tag="expz")

                    for s in range(NSUB):
                        c0 = s * SUB
                        # v.T = relu(W_video V.T + b) : [h, rows]
                        for m in range(HC):
                            v_ps = mm_ps.tile([128, SUB], f32, tag="v_ps")
                            for k in range(KC):
                                nc.tensor.matmul(
                                    v_ps,
                                    wvideoT_sb[:, k, m * 128 : (m + 1) * 128],
                                    vt_tile[:, k, c0 : c0 + SUB],
                                    start=(k == 0),
                                    stop=(k == KC - 1),
                                )
                            nc.scalar.activation(
                                out=vrelu[:, m, c0 : c0 + SUB], in_=v_ps, func=AF.Relu,
                                bias=bvideo_sb[:, m : m + 1],
                            )
                        # content.T = W_v v.T + 1^T inter : [48, rows]
                        c_ps = ct_ps.tile([MSIZE, SUB], f32, tag="c_ps")
                        for k in range(HC):
                            nc.tensor.matmul(
                                c_ps,
                                wvT_sb[:, k, :],
                                vrelu[:, k, c0 : c0 + SUB],
                                start=(k == 0),
                                stop=False,
                            )
                        nc.tensor.matmul(
                            c_ps,
                            ones48,
                            interflat[:, c0 : c0 + SUB],
                            start=False,
                            stop=True,
                        )
                        nc.scalar.activation(
                            out=tanhc[:, c0 : c0 + SUB], in_=c_ps, func=AF.Tanh
                        )
                        # z = W_h tanh(content).T : [1, rows]
                        zt_ps = z_ps.tile([1, SUB], f32, tag="zt_ps")
                        nc.tensor.matmul(
                            zt_ps,
                            whT_sb,
                            tanhc[:, c0 : c0 + SUB],
                            start=True,
                            stop=True,
                        )
                        nc.scalar.activation(
                            out=expz[:, c0 : c0 + SUB], in_=zt_ps, func=AF.Exp
                        )
                        # broadcast expz over 128 partitions via K=1 matmul
                        e_ps = eb_ps.tile([128, SUB], f32, tag="e_ps")
                        nc.tensor.matmul(
                            e_ps,
                            ones128,
                            expz[:, c0 : c0 + SUB],
                            start=True,
                            stop=True,
                        )
                        # weighted V.T (in place)
                        for c in range(KC):
                            nc.vector.tensor_mul(
                                vt_tile[:, c, c0 : c0 + SUB],
                                vt_tile[:, c, c0 : c0 + SUB],
                                e_ps,
                            )
                    # denominator partial: sum expz over each 48-group
                    nc.vector.reduce_sum(
                        out=denom_sb[:, sb * GPS : (sb + 1) * GPS],
                        in_=expz.rearrange("p (g n) -> p g n", n=MSIZE),
                        axis=AX.X,
                    )
                    # segmented sum of weighted V.T -> c.T columns
                    for c in range(KC):
                        nc.vector.reduce_sum(
                            out=cT_acc[:, c, sb * GPS : (sb + 1) * GPS],
                            in_=vt_tile[:, c, :].rearrange("p (g n) -> p g n", n=MSIZE),
                            axis=AX.X,
                        )

                # ---- tail: normalize and store ----
                with nc.allow_low_precision(reason="f32r (fp22) reciprocal is plenty"):
                    nc.vector.reciprocal(out=recip_sb, in_=denom_sb)
                r_ps = eb_ps.tile([128, PER], f32, tag="e_ps")
                nc.tensor.matmul(
                    r_ps, ones128, recip_sb,
                    start=True, stop=True,
                )
                for c in range(KC):
                    nc.vector.tensor_mul(cT_acc[:, c, :], cT_acc[:, c, :], r_ps)
                nc.sync.dma_start(
                    out=cT_d.rearrange("(c p) n -> p c n", p=128), in_=cT_acc
                )

    nc.compile()
    return nc


def _prep_in_maps(inputs):
    audio = np.ascontiguousarray(np.asarray(inputs["audio"], np.float32))
    video = np.ascontiguousarray(np.asarray(inputs["video"], np.float32))
    WvideoT = np.ascontiguousarray(np.asarray(inputs["W_video"], np.float32).T)
    WaudioT = np.ascontiguousarray(np.asarray(inputs["W_audio"], np.float32).T)
    WgT = np.ascontiguousarray(np.asarray(inputs["W_g"], np.float32).T)
    WvT = np.ascontiguousarray(np.asarray(inputs["W_v"], np.float32).T)
    WhT = np.ascontiguousarray(np.asarray(inputs["W_h"], np.float32).T)
    b_video = np.ascontiguousarray(np.asarray(inputs["b_video"], np.float32))
    b_audio = np.ascontiguousarray(np.asarray(inputs["b_audio"], np.float32))

    a2 = audio.reshape(BT, ASIZE)
    v2 = video.reshape(BT, MSIZE, VSIZE)
    in_maps = []
    for c in range(NCORES):
        sl = slice(c * PER, (c + 1) * PER)
        vT = np.ascontiguousarray(v2[sl].reshape(R, VSIZE).T)
        audioT = np.ascontiguousarray(a2[sl].T)
        in_maps.append(
            {
                "vT": vT,
                "audioT": audioT,
                "WvideoT": WvideoT,
                "WaudioT": WaudioT,
                "WgT": WgT,
                "WvT": WvT,
                "WhT": WhT,
                "b_video": b_video,
                "b_audio": b_audio,
            }
        )
    return in_maps


def _run(inputs, trace=False, **spmd_kwargs):
    from concourse.bass_utils import run_bass_kernel_spmd

    if "nc" not in _cached:
        _cached["nc"] = _build_nc()
    nc = _cached["nc"]
    in_maps = _prep_in_maps(inputs)
    res = run_bass_kernel_spmd(
        nc, in_maps, core_ids=list(range(NCORES)), trace=trace, **spmd_kwargs
    )
    parts = [r["cT"].T for r in res.results]          # each [256, 512]
    out = np.concatenate(parts, axis=0).reshape(B, T, VSIZE)
    return np.ascontiguousarray(out.astype(np.float32)), res


def kernel(**inputs):
    out, _ = _run(inputs, trace=False)
    return out
